# revision 39
# baseline (speedup 1.0000x reference)
"""Trainium2 Bass kernel for nn_Attention_9594956939856.

Single-head spatial self-attention over 64x64 feature maps:
    q = Wq@x+bq, k = Wk@x+bk, v = Wv@x+bv  (1x1 convs over channels)
    out = gamma * softmax(q^T k) @ v + x

Sharding: data-parallel over batch — 8 samples onto 8 NeuronCores, each core
computes one full sample (C=256, N=4096 tokens, dk=32). No collectives.

Design (ScalarE/ACT-bound; everything else hidden under the exp stream —
TimelineSim 149.4us/core vs 172.9 for the previous version):
  - The 16.8M-element exp of the score matrix is the hard floor: only the
    ACT engine has exp, at 1 elem/lane/cycle -> ~133us busy. The kernel is
    organized so ACT runs back-to-back exps for its entire duration.
  - scores via the G-trick: s[j,i] = x_j^T G x_i with G = Wq^T Wk folded on
    the host. Removes q/k projections; the scores matmul contracts over
    C=256 channels, which exactly fills fp8 DoubleRow (0.5 cyc/row) using
    the same x8 pair-layout tile the V projection uses.
  - all PE inputs fp8 e4m3 with power-of-2 prescales for range:
    x8 = x/4, G8 = 1024*G (so psum = 64*s; exp applies scale=1/64),
    wv8 = 16*gamma*Wv (so the finalize is just recip + mult + add-residual;
    gamma=0 gives exactly y = x).
  - softmax denominator: rho8 (4*exp of the per-j bias row, ==4 when bq==0)
    replaces all-ones as the DR stationary, M=128 so the denominator is
    broadcast to all partitions for free. Single stage (no quad-sum).
  - per-j bias r_j = (Wk^T bq)@x_j (exact softmax decomposition: the per-i
    term and the constant are softmax-invariant and dropped) is applied by
    scaling vt8 and the denominator stationary by rho_j = exp(r_j).
  - software pipeline: AV/denominator matmuls for i-chunk c-1 are emitted
    interleaved with the scores+exp stream of chunk c; gk/r projections are
    interleaved into chunk 0, v-projection/vt8 prep into chunk 1; a single
    "boot" DMA carries x8-chunk0+G8+wr8 so the first exp fires ~6us in.
  - exp on ScalarE in (128,1024) chunks (fp8 out), fp32 PSUM accumulation.
    Scores are in [-5,5] for this input distribution, so softmax without
    max-subtraction is numerically safe.
"""

import ml_dtypes
import numpy as np

import concourse.bass as bass
import concourse.mybir as mybir
from concourse.tile import TileContext
from concourse.bass_utils import run_bass_kernel_spmd

B, C, H, W = 8, 256, 64, 64
N = H * W          # 4096 tokens
DK = C // 8        # 32
P = 128
F32 = mybir.dt.float32
FP8 = mybir.dt.float8e4
DR = mybir.MatmulPerfMode.DoubleRow
AF = mybir.ActivationFunctionType
ALU = mybir.AluOpType

NJT = N // P       # 32 j-tiles
NJP = NJT // 2     # 16 j-pairs
HCH = 512          # i-chunk width (one PSUM bank of fp32)
NCH = N // HCH     # 8 chunks


# ---------------------------------------------------------------------------
# Workaround: the walrus build in this container allows only ONE sync wait
# per instruction ("Too many sync wait commands"), but Tile's wait
# assignment attaches up to 2 (and the tail drain more). Hoist all-but-one
# wait of any over-subscribed instruction onto dedicated same-engine nofuse
# nops inserted immediately before it in the ordered stream.
_PATCHED = False


def _apply_tile_patch():
    global _PATCHED
    if _PATCHED:
        return
    from concourse.tile import TileContext as TC
    from concourse.vector_clock import ScopedClock, VectorClock

    def _drain_and_barrier_split(self, tick_clock, wait_clock):
        gc = tick_clock.global_clock
        n = len(gc)
        for i in range(n):
            if gc[i] > 0:
                vec = [0] * n
                vec[i] = gc[i]
                ins = self.nc.sync.nop(nofuse=True, hint="tail_drain_wait")
                wait_clock.add_sem_waits(
                    ins.ins, ScopedClock({None: VectorClock(vec)})
                )
        self.nc.sync.drain()
        self.nc.all_engine_barrier()
        assert self.sems is not None
        popped = self.nc._tile_sem_poison_stack.pop()
        assert popped is self._sem_poison
        self.nc.clear_and_free_semaphores(list(self.sems.allocated().values()))
        self.nc.all_engine_barrier()

    TC._drain_and_barrier = _drain_and_barrier_split

    orig_lower = TC._lower_ordered_insts
    counter = [0]

    def _lower_split_waits(self, ordered):
        for bb_name, insts in ordered.items():
            new = []
            changed = False
            for inst in insts:
                si = inst.sync_info
                if si is not None and len(si.on_wait) > 1:
                    changed = True
                    waits = list(si.on_wait)
                    for w in waits[:-1]:
                        counter[0] += 1
                        new.append(
                            mybir.InstNoOp(
                                name=f"splitw-{counter[0]}",
                                sync_info=mybir.SyncInfo(
                                    on_wait=[w], on_update=[]
                                ),
                                bass_nofuse=True,
                                engine=inst.engine,
                            )
                        )
                    inst.sync_info = mybir.SyncInfo(
                        on_wait=[waits[-1]], on_update=list(si.on_update)
                    )
                new.append(inst)
            if changed:
                insts[:] = new
        return orig_lower(self, ordered)

    TC._lower_ordered_insts = _lower_split_waits
    _PATCHED = True


def _emit_body(nc, tc, pools, ext):
    """Emit one full attention computation (one sample)."""
    consts, big, epool, fin, ps_s_pool, ps_acc_pool = pools
    x_e, x8_e, boot_e, wv8_e, bv4_e, y_e = ext

    # ---- constants / weights ---------------------------------------------
    # boot: one DMA carrying [x8-o0-ch0 | x8-o1-ch0 | g8 | wr8] so the
    # whole critical lead-in rides a single transfer
    boot_t = consts.tile([P, 2 * HCH + 2 * C + 2], FP8, tag="boot_t")
    wv8_t = consts.tile([P, 2 * C], FP8, tag="wv8_t")
    bv4_t = consts.tile([P, C], F32, tag="bv4_t")
    # rho8 = 4*rho: the 4 makes recip(pd) directly the 1/(4*sum) the
    # finalize needs (gamma itself is folded into wv8/bv4 on the host)
    four_f = consts.tile([P, P], F32, tag="four_f")

    nc.vector.memset(four_f[:], 4.0)

    x8b = boot_t[:, 0 : 2 * HCH].rearrange("p (o i) -> p o i", o=2)
    g8r = boot_t[:, 2 * HCH : 2 * HCH + 2 * C].rearrange(
        "p (o m) -> p o m", o=2
    )                                                    # c=(o,p); m = c' half
    wr8r = boot_t[:, 2 * HCH + 2 * C :].rearrange("p (o m) -> p o m", o=2)
    wv8r = wv8_t[:].rearrange("p (o m) -> p o m", o=2)   # c=(o,p); m = c'

    xf0 = big.tile([P, N], F32, tag="xf0")
    xf1 = big.tile([P, N], F32, tag="xf1")
    x8 = big.tile([P, 2 * N], FP8, tag="x8")     # [p, o*N + i]: x[o*128+p, i]/4
    gk8 = big.tile([P, 2 * N], FP8, tag="gk8")   # [p, o*N + j]: 256*(G^T x)[o*128+p, j]
    # vt8 pair layout (as baseline): [p, jp*512 + h*256 + o*128 + m]
    #   = 4*rho_j*(v+bv)[c = h*128+m, j = jp*256 + o*128 + p]
    # +128 pad: the strided (h,z) store AP for the last j-tile spans 512
    # columns from base 7808 even though only blocks {0,256}+128 are written
    vt8 = big.tile([P, NJP * 512 + P, ], FP8, tag="vt8")
    # rho8: [p, jt*128 + m] = rho[j = jt*128 + p] (replicated over m)
    rho8 = big.tile([P, NJT * P], FP8, tag="rho8")
    rho_t = fin.tile([P, NJT], F32, tag="rho_t", bufs=1)

    x8r = x8[:].rearrange("p (o i) -> p o i", o=2)
    gk8r = gk8[:].rearrange("p (o j) -> p o j", o=2)

    # preload the Exp activation table while DMAs are in flight (the first
    # real exp would otherwise pay the ~1.3us table load on the critical path)
    warm = fin.tile([P, 1], F32, tag="warm", bufs=1)
    nc.vector.memset(warm[:], 0.0)
    nc.scalar.activation(warm[:], warm[:], AF.Exp)



    # The DMA engine pool serializes transfers, so queue order IS the
    # critical path: boot (x8 chunk 0 + G8 + wr8, one transfer) unblocks
    # the first scores; x8 bulk follows; residual x last.
    nc.sync.dma_start(out=boot_t[:], in_=boot_e[:])
    nc.sync.dma_start(out=x8[:, HCH : 2 * HCH], in_=x8_e[:, HCH : 2 * HCH])
    nc.sync.dma_start(
        out=x8[:, N + HCH : N + 2 * HCH], in_=x8_e[:, N + HCH : N + 2 * HCH]
    )
    nc.sync.dma_start(out=x8[:, 2 * HCH : N], in_=x8_e[:, 2 * HCH : N])
    nc.sync.dma_start(
        out=x8[:, N + 2 * HCH : 2 * N], in_=x8_e[:, N + 2 * HCH : 2 * N]
    )
    nc.sync.dma_start(out=wv8_t[:], in_=wv8_e[:])
    nc.sync.dma_start(out=bv4_t[:], in_=bv4_e[:])
    # fp32 residual x — not needed before finalize(0) at ~35us
    for q in range(4):
        sl = slice(q * (N // 4), (q + 1) * (N // 4))
        nc.sync.dma_start(out=xf0[:, sl], in_=x_e[0:P, sl])
        nc.sync.dma_start(out=xf1[:, sl], in_=x_e[P : 2 * P, sl])

    def x8v(sl):
        """x8 pair view for an absolute column slice; chunk 0 lives in boot."""
        return x8b[:, :, sl] if sl.stop <= HCH else x8r[:, :, sl]

    # pr shares the "pd" slot (same shape); only the first NJT cols are used
    pr = ps_acc_pool.tile([P, HCH], F32, tag="pd", bufs=1)

    def emit_gkchunk(ch):
        """r + gk projections for one 512-wide j-chunk."""
        sl = slice(ch * HCH, (ch + 1) * HCH)
        for jt in range(4 * ch, 4 * ch + 4):
            jsl = slice(jt * P, (jt + 1) * P)
            nc.tensor.matmul(
                pr[:, jt : jt + 1], x8v(jsl), wr8r,
                start=True, stop=True, perf_mode=DR,
            )
        for h in range(2):
            pg = ps_acc_pool.tile(
                [P, HCH], F32, tag="po0" if h == 0 else "po1", bufs=1,
            )
            nc.tensor.matmul(
                pg[:], g8r[:, :, h * P : (h + 1) * P], x8v(sl),
                start=True, stop=True, perf_mode=DR,
            )
            nc.vector.tensor_copy(gk8r[:, h, sl], pg[:])

    def emit_vprep(jt):
        """v projection + vt8 pair-layout store (rho folded), rho8 build."""
        jsl = slice(jt * P, (jt + 1) * P)
        pv = ps_acc_pool.tile([P, C], F32, tag="pv", bufs=1)
        nc.tensor.matmul(
            pv[:], x8v(jsl), wv8r, start=True, stop=True, perf_mode=DR,
        )
        jp, o = jt // 2, jt % 2
        base = jp * 512 + o * P
        # this jt owns 128-blocks at base (h=0) and base+256 (h=1)
        out_ap = vt8[:, base : base + 512].rearrange(
            "p (h z m) -> p h z m", h=2, z=2
        )[:, :, 0, :]
        nc.vector.tensor_tensor(
            out_ap, pv[:].rearrange("p (h m) -> p h m", h=2),
            bv4_t[:].rearrange("p (h m) -> p h m", h=2), op=ALU.add,
        )
        nc.vector.tensor_scalar_mul(out_ap, out_ap, rho_t[:, jt : jt + 1])
        nc.vector.tensor_scalar_mul(
            rho8[:, jt * P : (jt + 1) * P], four_f[:], rho_t[:, jt : jt + 1]
        )

    # ---- attention: software-pipelined over i-chunks ---------------------
    prev_e = None
    prev_isl = None

    def emit_av(jp, e_tiles, st, sp, po0, po1, pd):
        rhs = e_tiles[jp][:].rearrange("p (o i) -> p o i", o=2)
        for h, po in ((0, po0), (1, po1)):
            lhsT = vt8[
                :, jp * 512 + h * 2 * P : jp * 512 + (h + 1) * 2 * P
            ].rearrange("p (o m) -> p o m", o=2)
            nc.tensor.matmul(po[:], lhsT, rhs, start=st, stop=sp, perf_mode=DR)
        rl = rho8[:, jp * 2 * P : (jp + 1) * 2 * P].rearrange(
            "p (o m) -> p o m", o=2
        )
        nc.tensor.matmul(pd[:], rl, rhs, start=st, stop=sp, perf_mode=DR)

    def emit_finalize(isl, po0, po1, pd):
        dr = fin.tile([P, HCH], F32, tag="dr")
        nc.vector.reciprocal(dr[:], pd[:])
        t0 = fin.tile([P, HCH], F32, tag="t0")
        nc.vector.tensor_tensor(t0[:], po0[:], dr[:], op=ALU.mult)
        nc.vector.tensor_tensor(t0[:], t0[:], xf0[:, isl], op=ALU.add)
        nc.sync.dma_start(out=y_e[0:P, isl], in_=t0[:])
        t1 = fin.tile([P, HCH], F32, tag="t1")
        nc.vector.tensor_tensor(t1[:], po1[:], dr[:], op=ALU.mult)
        nc.vector.tensor_tensor(t1[:], t1[:], xf1[:, isl], op=ALU.add)
        nc.sync.dma_start(out=y_e[P : 2 * P, isl], in_=t1[:])

    for ich in range(NCH + 1):
        if ich < NCH:
            isl = slice(ich * HCH, (ich + 1) * HCH)
            cur_e = []
        if prev_e is not None:
            po0 = ps_acc_pool.tile([P, HCH], F32, tag="po0", bufs=1)
            po1 = ps_acc_pool.tile([P, HCH], F32, tag="po1", bufs=1)
            pd = ps_acc_pool.tile([P, HCH], F32, tag="pd", bufs=1)
        if ich == 0:
            emit_gkchunk(0)
        for jp in range(NJP):
            if ich < NCH:
                ps = ps_s_pool.tile([P, 2 * HCH], F32, tag="ps_s", bufs=2)
                for o in range(2):
                    jt = 2 * jp + o
                    nc.tensor.matmul(
                        ps[:, o * HCH : (o + 1) * HCH],
                        gk8r[:, :, jt * P : (jt + 1) * P],
                        x8v(isl),
                        start=True, stop=True, perf_mode=DR,
                    )
                e8 = epool.tile([P, 2 * HCH], FP8, tag="e", bufs=34)
                nc.scalar.activation(e8[:], ps[:], AF.Exp, scale=1.0 / 64.0)
                cur_e.append(e8)
            if ich == 0 and jp % 2 == 0 and jp // 2 + 1 < NCH:
                # gk/r projections one chunk ahead of the scores needing
                # them (after this pair's scores so they don't block them)
                emit_gkchunk(jp // 2 + 1)
            if ich == 1:
                # rho_t is ready (emitted at end of ich 0); produce the
                # rho-scaled vt8 pair + rho8 stationary just ahead of the
                # AV(0) matmuls that consume them
                emit_vprep(2 * jp)
                emit_vprep(2 * jp + 1)
            if prev_e is not None:
                emit_av(jp, prev_e, jp == 0, jp == NJP - 1, po0, po1, pd)
        if ich == 0:
            # per-j softmax bias: rho = exp(r); tiny ACT op between chunks
            nc.scalar.activation(rho_t[:], pr[:, 0:NJT], AF.Exp)
        if prev_e is not None:
            emit_finalize(prev_isl, po0, po1, pd)
        if ich < NCH:
            prev_e, prev_isl = cur_e, isl


def build_bass(loop_n: int | None = None) -> bass.Bass:
    """Build the kernel. loop_n wraps the body in a device-side For_i loop
    (with a tiny 'tick' sentinel output) for slope-based benchmarking."""
    _apply_tile_patch()
    nc = bass.Bass()

    x_e = nc.declare_dram_parameter("x", [C, N], F32, isOutput=False)
    x8_e = nc.declare_dram_parameter("x8", [P, 2 * N], FP8, isOutput=False)
    boot_e = nc.declare_dram_parameter(
        "boot", [P, 2 * HCH + 2 * C + 2], FP8, isOutput=False
    )
    wv8_e = nc.declare_dram_parameter("wv8", [P, 2 * C], FP8, isOutput=False)
    bv4_e = nc.declare_dram_parameter("bv4", [P, C], F32, isOutput=False)
    y_e = nc.declare_dram_parameter("y", [C, N], F32, isOutput=True)
    tick_e = None
    if loop_n is not None:
        tick_e = nc.declare_dram_parameter("tick", [1, 8], F32, isOutput=True)

    ext = (x_e, x8_e, boot_e, wv8_e, bv4_e, y_e)

    with (
        TileContext(nc) as tc,
        tc.tile_pool(name="consts", bufs=1) as consts,
        tc.tile_pool(name="big", bufs=1) as big,
        tc.tile_pool(name="epool", bufs=34) as epool,
        tc.tile_pool(name="fin", bufs=2) as fin,
        tc.tile_pool(name="ps_s", bufs=1, space="PSUM") as ps_s_pool,
        tc.tile_pool(name="ps_acc", bufs=1, space="PSUM") as ps_acc_pool,
    ):
        pools = (consts, big, epool, fin, ps_s_pool, ps_acc_pool)
        if loop_n is None:
            _emit_body(nc, tc, pools, ext)
        else:
            with tc.For_i(0, loop_n, 1):
                _emit_body(nc, tc, pools, ext)
            t = fin.tile([1, 8], F32, tag="tick")
            nc.vector.memset(t[:], 1.0)
            nc.sync.dma_start(out=tick_e[:], in_=t[:])

    return nc


_NC_CACHE = None


def _get_nc() -> bass.Bass:
    global _NC_CACHE
    if _NC_CACHE is None:
        _NC_CACHE = build_bass()
    return _NC_CACHE


FP8NP = ml_dtypes.float8_e4m3


def _pair_layout(m):
    """(256, K) -> (128, 2*K) with [p, o*K + k] = m[o*128 + p, k]."""
    K = m.shape[1]
    return np.ascontiguousarray(
        m.reshape(2, 128, K).transpose(1, 0, 2).reshape(128, 2 * K)
    )


def prep_core_inputs(x, Wq, bq, Wk, bk, Wv, bv, gamma):
    x = np.asarray(x, np.float64).reshape(B, C, N)
    Wq = np.asarray(Wq, np.float64)
    Wk = np.asarray(Wk, np.float64)
    Wv = np.asarray(Wv, np.float64)
    bq = np.asarray(bq, np.float64)
    bv = np.asarray(bv, np.float64)
    gamma = float(np.asarray(gamma).reshape(-1)[0])

    G = Wq.T @ Wk                    # (C, C); s[j,i] = x_j^T G x_i (+ row/col terms)
    g8 = _pair_layout(1024.0 * G).astype(FP8NP)
    wv8 = _pair_layout(16.0 * gamma * Wv.T).astype(FP8NP)
    wr = (Wk.T @ bq).reshape(C, 1)   # per-j bias direction
    wr8 = _pair_layout(4.0 * wr).astype(FP8NP)
    bv4 = np.ascontiguousarray(
        np.broadcast_to(4.0 * gamma * bv, (P, C))
    ).astype(np.float32)

    shared = {"wv8": wv8, "bv4": bv4}
    maps = []
    for b in range(B):
        x8 = _pair_layout(0.25 * x[b]).astype(FP8NP)
        boot = np.concatenate(
            [
                x8[:, 0:HCH], x8[:, N : N + HCH],  # chunk 0, both planes
                g8, wr8,
            ],
            axis=1,
        )
        maps.append(
            {
                "x": np.ascontiguousarray(x[b]).astype(np.float32),
                "x8": x8,
                "boot": np.ascontiguousarray(boot),
                **shared,
            }
        )
    return maps


def kernel(**inputs) -> np.ndarray:
    nc = _get_nc()
    in_maps = prep_core_inputs(**inputs)
    res = run_bass_kernel_spmd(nc, in_maps, list(range(B)))
    y = np.stack([res.results[i]["y"] for i in range(B)])
    return np.ascontiguousarray(y.reshape(B, C, H, W).astype(np.float32))
